# revision 15
# baseline (speedup 1.0000x reference)
"""Trainium2 Bass kernel for a dense transformer block (pre-LN, 12 heads, MLP 4x).

Strategy: data-parallel over batch across the 8 NeuronCores (B=8 -> one batch
element per core, no collectives). Per core, v2 (fp8 DoubleRow):

  - residual stream token-major fp32 [128 tok x 768] (8 token chunks)
  - LN on DVE via bn_stats/bn_aggr; LN affine params folded into the weights
  - h (LN1 out) stored feature-major in e4m3; QKV/V/proj/fc2 matmuls run in
    fp8 DoubleRow mode (contraction pairs packed in the free dim -> 2x K per
    pass); S and fc1 stay bf16 for accuracy (rel-err budget).
  - attention computed transposed: S_t[k,q] = k_fm.T @ q_fm, head pairs in
    disjoint PE row groups; exp on ACT with output scaled by 2^EXP_K (folded
    into the exp bias) and stored e4m3 so the ctx matmul can run DoubleRow;
    the 2^EXP_K cancels in the softmax normalization.
  - fc2 weights scaled by 64 on host (fp8 subnormal avoidance); descale is
    fused into the PSUM evacuation.
"""

from contextlib import ExitStack

import numpy as np

import concourse.bass as bass
import concourse.mybir as mybir
import concourse.tile as tile
from concourse import bacc
from concourse.masks import make_identity

DIM = 768
HEADS = 12
HD = 64  # head dim
HIDDEN = 3072
N_TOK = 1024
TC = N_TOK // 128  # 8 token chunks
FC = DIM // 128  # 6 feature chunks
MC_H = HIDDEN // 128  # 24 hidden chunks
EPS = 1e-5
SCALE = HD ** -0.5
EXP_K = 4  # exps scaled by 2^EXP_K (cancels in softmax norm)
FC2_WS = 64.0  # fc2 weight scale (descale fused in evacuation)
FILLER = 5  # keep-warm matmuls per S/exp group
VPAD = 80  # padded per-(head,chunk) v stride, 16B-aligned for DoubleRow APs

F32 = mybir.dt.float32
BF16 = mybir.dt.bfloat16
F8 = mybir.dt.float8e4
DR = mybir.MatmulPerfMode.DoubleRow


def _ln_chunk(nc, stat_pool, eps_tile, x_ap, out_ap):
    """out = (x - mean(x)) * rsqrt(var(x) + eps), row-wise over 768."""
    stats = stat_pool.tile([128, 3, 6], F32, tag="ln_stats")
    for sg in range(3):
        nc.vector.bn_stats(out=stats[:, sg, :], in_=x_ap[:, sg * 256:(sg + 1) * 256])
    mv = stat_pool.tile([128, 2], F32, tag="ln_mv")
    nc.vector.bn_aggr(out=mv, in_=stats)
    rstd = stat_pool.tile([128, 1], F32, tag="ln_rstd")
    nc.scalar.activation(
        out=rstd, in_=mv[:, 1:2], func=mybir.ActivationFunctionType.Sqrt,
        bias=eps_tile, scale=1.0,
    )
    nc.vector.reciprocal(out=rstd, in_=rstd)
    nc.vector.tensor_scalar(
        out=out_ap, in0=x_ap, scalar1=mv[:, 0:1], scalar2=rstd,
        op0=mybir.AluOpType.subtract, op1=mybir.AluOpType.mult,
    )


class TileCtx:
    """TileContext + an ExitStack, flattened to dodge the nested-block limit."""

    def __init__(self, nc):
        self.st = ExitStack()
        self.nc = nc

    def __enter__(self):
        tc = self.st.enter_context(tile.TileContext(self.nc))
        return tc, self.st

    def __exit__(self, *exc):
        return self.st.__exit__(*exc)


class _Pools:
    NAMES = ()

    def __init__(self, tc):
        self.st = ExitStack()
        self.tc = tc

    def __enter__(self):
        return tuple(self.st.enter_context(self.tc.tile_pool(name=n, bufs=b))
                     for n, b in self.NAMES)

    def __exit__(self, *exc):
        return self.st.__exit__(*exc)


class AttnPools(_Pools):
    NAMES = (("qk", 1), ("vaug", 1), ("ctxfm", 1), ("wproj", 1))


class QkvPools(_Pools):
    NAMES = (("hfm", 1), ("wqkv", 3), ("wvp", 2), ("exps", 12))


def build_bass():
    nc = bacc.Bacc("TRN2", debug=False)

    x_d = nc.dram_tensor("x", [N_TOK, DIM], F32, kind="ExternalInput")
    qkv_wt_d = nc.dram_tensor("qkv_wt", [DIM, 3 * DIM], F8, kind="ExternalInput")
    qkb_pm_d = nc.dram_tensor("qkb_pm", [128, 2 * FC], F32, kind="ExternalInput")
    vb_d = nc.dram_tensor("vb", [DIM], F32, kind="ExternalInput")
    proj_wt_d = nc.dram_tensor("proj_wt", [DIM, DIM], F8, kind="ExternalInput")
    projb_d = nc.dram_tensor("projb", [DIM], F32, kind="ExternalInput")
    fc1_wt_d = nc.dram_tensor("fc1_wt", [DIM, HIDDEN], BF16, kind="ExternalInput")
    fc1b_pm_d = nc.dram_tensor("fc1b_pm", [128, MC_H], F32, kind="ExternalInput")
    fc2_wt_d = nc.dram_tensor("fc2_wt", [HIDDEN, DIM], F8, kind="ExternalInput")
    fc2b64_d = nc.dram_tensor("fc2b64", [DIM], F32, kind="ExternalInput")
    out_d = nc.dram_tensor("out", [N_TOK, DIM], F32, kind="ExternalOutput")

    x_dt = x_d.ap().rearrange("(t p) c -> p t c", p=128)
    out_dt = out_d.ap().rearrange("(t p) c -> p t c", p=128)
    # weight chunk views: [128 part of in-feat, in-chunk, out-col]
    qkv_w3 = qkv_wt_d.ap().rearrange("(ko p) n -> p ko n", p=128)
    proj_w3 = proj_wt_d.ap().rearrange("(ko p) n -> p ko n", p=128)
    fc1_w3 = fc1_wt_d.ap().rearrange("(ko p) n -> p ko n", p=128)
    fc2_w3 = fc2_wt_d.ap().rearrange("(ko p) n -> p ko n", p=128)

    def bcast128(ap_1d, n):
        return bass.AP(tensor=ap_1d.tensor, offset=ap_1d.offset,
                       ap=[[0, 128], [1, n]])

    with TileCtx(nc) as (tc, st):
        if True:
            const_pool = st.enter_context(tc.tile_pool(name="const", bufs=1))
            resid_pool = st.enter_context(tc.tile_pool(name="resid", bufs=1))
            stat_pool = st.enter_context(tc.tile_pool(name="stats", bufs=3))
            dsm_pool = st.enter_context(tc.tile_pool(name="dsm", bufs=2))
            # PSUM: big (S tiles [128,1024] f32 = 2 banks; fc1; transposes),
            # small 1-bank (qkv/v/proj/fc2), ctx [65,512]. 2*2+2+2 = 8 banks.
            psum_big = st.enter_context(
                tc.tile_pool(name="psum_big", bufs=2, space="PSUM"))
            psum_small = st.enter_context(
                tc.tile_pool(name="psum_small", bufs=2, space="PSUM"))
            psum_ctx = st.enter_context(
                tc.tile_pool(name="psum_ctx", bufs=2, space="PSUM"))
            h2fm_pool = st.enter_context(tc.tile_pool(name="h2fm", bufs=1))
            g_pool = st.enter_context(tc.tile_pool(name="gfm", bufs=1))
            wfc2_pool = st.enter_context(tc.tile_pool(name="wfc2", bufs=1))
            out_pool = st.enter_context(tc.tile_pool(name="outt", bufs=2))

            x_sb = resid_pool.tile([128, TC, DIM], F32)
            for t in range(TC):
                nc.sync.dma_start(out=x_sb[:, t, :], in_=x_dt[:, t, :])
            ident = const_pool.tile([128, 128], BF16)
            make_identity(nc, ident)
            eps_tile = const_pool.tile([128, 1], F32)
            nc.vector.memset(eps_tile, EPS)
            qkb_pm = const_pool.tile([128, 2 * FC], F32)
            nc.sync.dma_start(out=qkb_pm, in_=qkb_pm_d.ap())
            fc1b_pm = const_pool.tile([128, MC_H], F32)
            nc.sync.dma_start(out=fc1b_pm, in_=fc1b_pm_d.ap())
            vb_bc = const_pool.tile([128, DIM], F32)
            nc.sync.dma_start(out=vb_bc, in_=bcast128(vb_d.ap(), DIM))
            projb_bc = const_pool.tile([128, DIM], F32)
            nc.sync.dma_start(out=projb_bc, in_=bcast128(projb_d.ap(), DIM))
            fc2b_bc = const_pool.tile([128, DIM], F32)
            nc.sync.dma_start(out=fc2b_bc, in_=bcast128(fc2b64_d.ap(), DIM))
            ones_bf = const_pool.tile([128, 128], BF16)
            nc.vector.memset(ones_bf, 1.0)
            expk_tile = const_pool.tile([128, 1], F32)
            nc.vector.memset(expk_tile, float(EXP_K * np.log(2.0)))
            projb_row = const_pool.tile([1, DIM], BF16)
            nc.vector.tensor_copy(out=projb_row, in_=projb_bc[0:1, :])
            fc2b_row = const_pool.tile([1, DIM], BF16)
            nc.vector.tensor_copy(out=fc2b_row, in_=fc2b_bc[0:1, :])

            h2_fm = h2fm_pool.tile([128, FC, N_TOK], BF16, tag="hfm2")
            g_fm = g_pool.tile([128, MC_H, N_TOK], F8)
            wf2 = wfc2_pool.tile([128, MC_H, DIM], F8)

            def ln_transpose(t, dst_fm, evac_act=False):
                """LN of token chunk t + PE-transpose into dst_fm[:, :, t*128:]."""
                h_tile = stat_pool.tile([128, DIM], BF16, tag="h_tile")
                _ln_chunk(nc, stat_pool, eps_tile, x_sb[:, t, :], h_tile)
                tr = psum_big.tile([128, FC, 128], BF16, tag="big")
                for f in range(FC):
                    nc.tensor.transpose(
                        tr[:, f, :], h_tile[:, f * 128:(f + 1) * 128], ident)
                if evac_act:
                    nc.scalar.copy(
                        out=dst_fm[:, :, t * 128:(t + 1) * 128], in_=tr)
                else:
                    nc.vector.tensor_copy(
                        out=dst_fm[:, :, t * 128:(t + 1) * 128], in_=tr)

            wfc1_pool = st.enter_context(tc.tile_pool(name="wfc1", bufs=4))

            def fc1_half(half):
                """fc1 (bf16) for one q-half of the tokens + gelu -> g_fm."""
                for m in range(MC_H):
                    w1t = wfc1_pool.tile([128, FC, 128], BF16, tag="w1t")
                    nc.sync.dma_start(
                        out=w1t, in_=fc1_w3[:, :, m * 128:(m + 1) * 128])
                    ps = psum_small.tile([128, 512], F32, tag="sm")
                    for k in range(FC):
                        nc.tensor.matmul(
                            ps, w1t[:, k, :],
                            h2_fm[:, k, half * 512:(half + 1) * 512],
                            start=(k == 0), stop=(k == FC - 1))
                    nc.scalar.activation(
                        out=g_fm[:, m, half * 512:(half + 1) * 512], in_=ps,
                        func=mybir.ActivationFunctionType.Gelu,
                        bias=fc1b_pm[:, m:m + 1], scale=1.0)

            # ============ attention region (qkv + attention + proj) =========
            with AttnPools(tc) as (qk_pool, v_pool, ctx_pool, wproj_pool):
                qk_fm = qk_pool.tile([128, 2 * FC, N_TOK], BF16)
                # v: [128 key, head, key-chunk, 64+1(den)] padded to VPAD for
                # 16B-aligned DoubleRow pair strides
                v_aug = v_pool.tile([128, HEADS, TC, VPAD], F8)
                ctx_fm = ctx_pool.tile([128, FC, N_TOK], F8)
                wp = wproj_pool.tile([128, FC, DIM], F8)

                with QkvPools(tc) as (hfm_pool, wqkv_pool, wv_pool, exps_pool):
                    h_fm = hfm_pool.tile([128, FC, N_TOK], F8, tag="hfm")

                    # v weights resident: [128, 6, 384] x2
                    wv = [wv_pool.tile([128, FC, 384], F8, tag="wv",
                                       name=f"wv{i}") for i in range(2)]
                    for nv in range(2):
                        nc.sync.dma_start(
                            out=wv[nv],
                            in_=qkv_w3[:, :, 2 * DIM + nv * 384:
                                       2 * DIM + (nv + 1) * 384])

                    # LN1 + v per token chunk (v starts the PE early)
                    for t in range(TC):
                        ln_transpose(t, h_fm)
                        for nv in range(2):
                            ps = psum_small.tile([128, 384], F32, tag="sm")
                            for c in range(FC // 2):
                                nc.tensor.matmul(
                                    ps,
                                    h_fm[:, 2 * c:2 * c + 2,
                                         t * 128:(t + 1) * 128],
                                    wv[nv][:, 2 * c:2 * c + 2, :],
                                    start=(c == 0), stop=(c == FC // 2 - 1),
                                    perf_mode=DR)
                            nc.vector.tensor_tensor(
                                out=v_aug[:, nv * 6:(nv + 1) * 6, t, 0:HD],
                                in0=ps.rearrange("p (h d) -> p h d", d=HD),
                                in1=vb_bc[:, nv * 384:(nv + 1) * 384].rearrange(
                                    "p (h d) -> p h d", d=HD),
                                op=mybir.AluOpType.add)
                    nc.vector.memset(v_aug[:, :, :, HD], 1.0)

                    def emit_qk(m):
                        wt = wqkv_pool.tile([128, FC, 128], F8, tag="wqkv")
                        nc.sync.dma_start(
                            out=wt, in_=qkv_w3[:, :, m * 128:(m + 1) * 128])
                        for q in range(2):
                            ps = psum_small.tile([128, 512], F32, tag="sm")
                            for c in range(FC // 2):
                                nc.tensor.matmul(
                                    ps, wt[:, 2 * c:2 * c + 2, :],
                                    h_fm[:, 2 * c:2 * c + 2,
                                         q * 512:(q + 1) * 512],
                                    start=(c == 0), stop=(c == FC // 2 - 1),
                                    perf_mode=DR)
                            nc.vector.tensor_scalar_add(
                                out=qk_fm[:, m, q * 512:(q + 1) * 512], in0=ps,
                                scalar1=qkb_pm[:, m:m + 1])

                    def emit_warm(dummy_ps, n):
                        """Keep-warm PE ops: HAM re-throttles the PE to
                        1.2 GHz after ~3.4us idle; the exp phase is ACT-bound
                        so real PE work alone leaves such gaps. ldweights
                        exercises the array without touching PSUM."""
                        for _ in range(n):
                            nc.tensor.ldweights(weights=ident)

                    def emit_s_exp(j):
                        """S + exp for head pair (2j, 2j+1); returns exps tiles
                        exps[ab][kcp] = [128, 2, N_TOK] e4m3 (kc pairs)."""
                        dummy_ps = None
                        exps = [[None] * (TC // 2) for _ in range(2)]
                        for kcp in range(TC // 2):
                            for ab in range(2):
                                e_t = exps_pool.tile([128, 2, N_TOK], F8,
                                                     tag="exps")
                                exps[ab][kcp] = e_t
                                po = 64 * ab
                                for sub in range(2):
                                    kc = 2 * kcp + sub
                                    sp = psum_big.tile([128, N_TOK], F32,
                                                       tag="big")
                                    for q in range(2):
                                        nc.tensor.matmul(
                                            sp[:, q * 512:(q + 1) * 512],
                                            qk_fm[po:po + 64, 6 + j,
                                                  kc * 128:(kc + 1) * 128],
                                            qk_fm[po:po + 64, j,
                                                  q * 512:(q + 1) * 512],
                                            start=True, stop=True)
                                    nc.scalar.activation(
                                        out=e_t[:, sub, :], in_=sp,
                                        func=mybir.ActivationFunctionType.Exp,
                                        scale=SCALE, bias=expk_tile)
                                emit_warm(dummy_ps, FILLER)
                        return exps

                    def emit_ctx(j, exps):
                        for ab in range(2):
                            h = 2 * j + ab
                            po = 64 * ab
                            for q in range(2):
                                cp = psum_ctx.tile([65, 512], F32, tag="ctx")
                                for kcp in range(TC // 2):
                                    nc.tensor.matmul(
                                        cp,
                                        v_aug[:, h, 2 * kcp:2 * kcp + 2, 0:65],
                                        exps[ab][kcp][:, :,
                                                      q * 512:(q + 1) * 512],
                                        start=(kcp == 0),
                                        stop=(kcp == TC // 2 - 1),
                                        perf_mode=DR)
                                den = dsm_pool.tile([1, 512], F32, tag="den")
                                nc.vector.tensor_copy(out=den,
                                                      in_=cp[64:65, :])
                                scr = dsm_pool.tile([1, 512], F32, tag="scr")
                                rec = dsm_pool.tile([1, 512], F32, tag="rec")
                                nc.vector.reciprocal_approx_accurate(
                                    out=rec, in_=den, scratch=scr)
                                bcd = dsm_pool.tile([128, 512], F32, tag="bcd")
                                nc.gpsimd.partition_broadcast(bcd, rec)
                                if ab == 0:
                                    nc.vector.tensor_tensor(
                                        out=ctx_fm[0:64, j,
                                                   q * 512:(q + 1) * 512],
                                        in0=cp[0:64, :], in1=bcd[0:64, :],
                                        op=mybir.AluOpType.mult)
                                else:
                                    cu = dsm_pool.tile([128, 512], F32,
                                                       tag="cu")
                                    nc.vector.tensor_copy(out=cu[po:po + 64, :],
                                                          in_=cp[0:64, :])
                                    nc.vector.tensor_tensor(
                                        out=ctx_fm[po:po + 64, j,
                                                   q * 512:(q + 1) * 512],
                                        in0=cu[po:po + 64, :],
                                        in1=bcd[po:po + 64, :],
                                        op=mybir.AluOpType.mult)

                    pend = None
                    for j in range(6):
                        emit_qk(j)
                        emit_qk(6 + j)
                        if j == 3:
                            nc.sync.dma_start(out=wp, in_=proj_w3)
                        if j == 4:
                            nc.sync.dma_start(out=wf2, in_=fc2_w3)
                        exps = emit_s_exp(j)
                        if pend is not None:
                            emit_ctx(*pend)
                        pend = (j, exps)
                    emit_ctx(*pend)

                # ---------------- proj + residual + LN2 ----------------
                for t in range(TC):
                    for nv in range(2):
                        ps = psum_small.tile([128, 384], F32, tag="sm")
                        for c in range(FC // 2):
                            nc.tensor.matmul(
                                ps,
                                ctx_fm[:, 2 * c:2 * c + 2,
                                       t * 128:(t + 1) * 128],
                                wp[:, 2 * c:2 * c + 2,
                                   nv * 384:(nv + 1) * 384],
                                start=(c == 0), stop=False, perf_mode=DR)
                        sl = slice(nv * 384, (nv + 1) * 384)
                        nc.tensor.matmul(
                            ps, ones_bf[0:1, :], projb_row[0:1, sl],
                            start=False, stop=True)
                        nc.vector.tensor_add(
                            out=x_sb[:, t, sl], in0=ps, in1=x_sb[:, t, sl])
                    ln_transpose(t, h2_fm, evac_act=True)
                    if t == 3:
                        fc1_half(0)
            fc1_half(1)

            # ---------------- MLP: fc2 (fp8 DR) ----------------
            if True:
                for t in range(TC):
                    o_t = out_pool.tile([128, DIM], F32, tag="outt")
                    for nv in range(2):
                        ps = psum_small.tile([128, 384], F32, tag="sm")
                        for c in range(MC_H // 2):
                            nc.tensor.matmul(
                                ps,
                                g_fm[:, 2 * c:2 * c + 2,
                                     t * 128:(t + 1) * 128],
                                wf2[:, 2 * c:2 * c + 2,
                                    nv * 384:(nv + 1) * 384],
                                start=(c == 0), stop=False, perf_mode=DR)
                        sl = slice(nv * 384, (nv + 1) * 384)
                        nc.tensor.matmul(
                            ps, ones_bf[0:1, :], fc2b_row[0:1, sl],
                            start=False, stop=True)
                        nc.vector.tensor_scalar_mul(
                            out=o_t[:, sl], in0=ps, scalar1=1.0 / FC2_WS)
                    nc.vector.tensor_add(out=o_t, in0=o_t, in1=x_sb[:, t, :])
                    nc.sync.dma_start(out=out_dt[:, t, :], in_=o_t)

    nc.compile()
    return nc


def host_prep(x, ln1_g, ln1_b, qkv_w, proj_w, proj_b, ln2_g, ln2_b,
              fc1_w, fc1_b, fc2_w, fc2_b):
    """Fold LN affine params into weights, pre-transpose, cast to fp8/bf16."""
    import ml_dtypes
    f32 = np.float32
    bf16 = ml_dtypes.bfloat16
    f8 = ml_dtypes.float8_e4m3
    qkv_w = np.asarray(qkv_w, f32)
    qkv_wt = np.ascontiguousarray(
        (qkv_w * np.asarray(ln1_g, f32)[None, :]).T).astype(f8)
    qkv_bias = qkv_w @ np.asarray(ln1_b, f32)
    qkb_pm = np.ascontiguousarray(qkv_bias[:2 * DIM].reshape(2 * FC, 128).T)
    vb = np.ascontiguousarray(qkv_bias[2 * DIM:])
    proj_wt = np.ascontiguousarray(np.asarray(proj_w, f32).T).astype(f8)
    fc1_w = np.asarray(fc1_w, f32)
    fc1_wt = np.ascontiguousarray(
        (fc1_w * np.asarray(ln2_g, f32)[None, :]).T).astype(bf16)
    fc1_bias = fc1_w @ np.asarray(ln2_b, f32) + np.asarray(fc1_b, f32)
    fc1b_pm = np.ascontiguousarray(fc1_bias.reshape(MC_H, 128).T)
    fc2_wt = np.ascontiguousarray(
        np.asarray(fc2_w, f32).T * FC2_WS).astype(f8)
    fc2b64 = np.ascontiguousarray(np.asarray(fc2_b, f32) * FC2_WS)
    return {
        "qkv_wt": qkv_wt, "qkb_pm": qkb_pm, "vb": vb,
        "proj_wt": proj_wt, "projb": np.ascontiguousarray(np.asarray(proj_b, f32)),
        "fc1_wt": fc1_wt, "fc1b_pm": fc1b_pm,
        "fc2_wt": fc2_wt, "fc2b64": fc2b64,
    }


_CACHE = {}


def kernel(x, ln1_g, ln1_b, qkv_w, proj_w, proj_b, ln2_g, ln2_b,
           fc1_w, fc1_b, fc2_w, fc2_b, _want_results=False, **_ignored):
    from concourse.bass_utils import run_bass_kernel_spmd

    x = np.asarray(x, np.float32)
    B = x.shape[0]
    assert B == 8 and x.shape[1] == N_TOK and x.shape[2] == DIM

    w = host_prep(x, ln1_g, ln1_b, qkv_w, proj_w, proj_b, ln2_g, ln2_b,
                  fc1_w, fc1_b, fc2_w, fc2_b)

    if "nc" not in _CACHE:
        _CACHE["nc"] = build_bass()
    nc = _CACHE["nc"]

    in_maps = [dict(w, x=np.ascontiguousarray(x[i])) for i in range(B)]
    res = run_bass_kernel_spmd(nc, in_maps, core_ids=list(range(B)))
    out = np.stack([res.results[i]["out"] for i in range(B)], axis=0)
    if _want_results:
        return out, res
    return out


# revision 16
# speedup vs baseline: 1.0435x; 1.0435x over previous
"""Trainium2 Bass kernel for a dense transformer block (pre-LN, 12 heads, MLP 4x).

Strategy: data-parallel over batch across the 8 NeuronCores (B=8 -> one batch
element per core, no collectives). Per core, v2 (fp8 DoubleRow):

  - residual stream token-major fp32 [128 tok x 768] (8 token chunks)
  - LN on DVE via bn_stats/bn_aggr; LN affine params folded into the weights
  - h (LN1 out) stored feature-major in e4m3; QKV/V/proj/fc2 matmuls run in
    fp8 DoubleRow mode (contraction pairs packed in the free dim -> 2x K per
    pass); S and fc1 stay bf16 for accuracy (rel-err budget).
  - attention computed transposed: S_t[k,q] = k_fm.T @ q_fm, head pairs in
    disjoint PE row groups; exp on ACT with output scaled by 2^EXP_K (folded
    into the exp bias) and stored e4m3 so the ctx matmul can run DoubleRow;
    the 2^EXP_K cancels in the softmax normalization.
  - fc2 weights scaled by 64 on host (fp8 subnormal avoidance); descale is
    fused into the PSUM evacuation.
"""

from contextlib import ExitStack

import numpy as np

import concourse.bass as bass
import concourse.mybir as mybir
import concourse.tile as tile
from concourse import bacc
from concourse.masks import make_identity

DIM = 768
HEADS = 12
HD = 64  # head dim
HIDDEN = 3072
N_TOK = 1024
TC = N_TOK // 128  # 8 token chunks
FC = DIM // 128  # 6 feature chunks
MC_H = HIDDEN // 128  # 24 hidden chunks
EPS = 1e-5
SCALE = HD ** -0.5
EXP_K = 4  # exps scaled by 2^EXP_K (cancels in softmax norm)
FC2_WS = 64.0  # fc2 weight scale (descale fused in evacuation)
FILLER = 5  # keep-warm matmuls per S/exp group
VPAD = 80  # padded per-(head,chunk) v stride, 16B-aligned for DoubleRow APs

F32 = mybir.dt.float32
BF16 = mybir.dt.bfloat16
F8 = mybir.dt.float8e4
DR = mybir.MatmulPerfMode.DoubleRow


def _ln_chunk(nc, stat_pool, eps_tile, x_ap, out_ap):
    """out = (x - mean(x)) * rsqrt(var(x) + eps), row-wise over 768."""
    stats = stat_pool.tile([128, 3, 6], F32, tag="ln_stats")
    for sg in range(3):
        nc.vector.bn_stats(out=stats[:, sg, :], in_=x_ap[:, sg * 256:(sg + 1) * 256])
    mv = stat_pool.tile([128, 2], F32, tag="ln_mv")
    nc.vector.bn_aggr(out=mv, in_=stats)
    rstd = stat_pool.tile([128, 1], F32, tag="ln_rstd")
    nc.scalar.activation(
        out=rstd, in_=mv[:, 1:2], func=mybir.ActivationFunctionType.Sqrt,
        bias=eps_tile, scale=1.0,
    )
    nc.vector.reciprocal(out=rstd, in_=rstd)
    nc.vector.tensor_scalar(
        out=out_ap, in0=x_ap, scalar1=mv[:, 0:1], scalar2=rstd,
        op0=mybir.AluOpType.subtract, op1=mybir.AluOpType.mult,
    )


class TileCtx:
    """TileContext + an ExitStack, flattened to dodge the nested-block limit."""

    def __init__(self, nc):
        self.st = ExitStack()
        self.nc = nc

    def __enter__(self):
        tc = self.st.enter_context(tile.TileContext(self.nc))
        return tc, self.st

    def __exit__(self, *exc):
        return self.st.__exit__(*exc)


class _Pools:
    NAMES = ()

    def __init__(self, tc):
        self.st = ExitStack()
        self.tc = tc

    def __enter__(self):
        return tuple(self.st.enter_context(self.tc.tile_pool(name=n, bufs=b))
                     for n, b in self.NAMES)

    def __exit__(self, *exc):
        return self.st.__exit__(*exc)


class AttnPools(_Pools):
    NAMES = (("qk", 1), ("vaug", 1), ("ctxfm", 1), ("wproj", 1))


class QkvPools(_Pools):
    NAMES = (("hfm", 1), ("wqkv", 3), ("wvp", 2), ("exps", 12))


def build_bass():
    nc = bacc.Bacc("TRN2", debug=False)

    x_d = nc.dram_tensor("x", [N_TOK, DIM], F32, kind="ExternalInput")
    qkv_wt_d = nc.dram_tensor("qkv_wt", [DIM, 3 * DIM], F8, kind="ExternalInput")
    qkb_pm_d = nc.dram_tensor("qkb_pm", [128, 2 * FC], F32, kind="ExternalInput")
    vb_d = nc.dram_tensor("vb", [DIM], F32, kind="ExternalInput")
    proj_wt_d = nc.dram_tensor("proj_wt", [DIM, DIM], F8, kind="ExternalInput")
    projb_d = nc.dram_tensor("projb", [DIM], F32, kind="ExternalInput")
    fc1_wt_d = nc.dram_tensor("fc1_wt", [DIM, HIDDEN], BF16, kind="ExternalInput")
    fc1b_pm_d = nc.dram_tensor("fc1b_pm", [128, MC_H], F32, kind="ExternalInput")
    fc2_wt_d = nc.dram_tensor("fc2_wt", [HIDDEN, DIM], F8, kind="ExternalInput")
    fc2b64_d = nc.dram_tensor("fc2b64", [DIM], F32, kind="ExternalInput")
    out_d = nc.dram_tensor("out", [N_TOK, DIM], F32, kind="ExternalOutput")

    x_dt = x_d.ap().rearrange("(t p) c -> p t c", p=128)
    out_dt = out_d.ap().rearrange("(t p) c -> p t c", p=128)
    # weight chunk views: [128 part of in-feat, in-chunk, out-col]
    qkv_w3 = qkv_wt_d.ap().rearrange("(ko p) n -> p ko n", p=128)
    proj_w3 = proj_wt_d.ap().rearrange("(ko p) n -> p ko n", p=128)
    fc1_w3 = fc1_wt_d.ap().rearrange("(ko p) n -> p ko n", p=128)
    fc2_w3 = fc2_wt_d.ap().rearrange("(ko p) n -> p ko n", p=128)

    def bcast128(ap_1d, n):
        return bass.AP(tensor=ap_1d.tensor, offset=ap_1d.offset,
                       ap=[[0, 128], [1, n]])

    with TileCtx(nc) as (tc, st):
        if True:
            const_pool = st.enter_context(tc.tile_pool(name="const", bufs=1))
            resid_pool = st.enter_context(tc.tile_pool(name="resid", bufs=1))
            stat_pool = st.enter_context(tc.tile_pool(name="stats", bufs=3))
            dsm_pool = st.enter_context(tc.tile_pool(name="dsm", bufs=2))
            # PSUM: big (S tiles [128,1024] f32 = 2 banks; fc1; transposes),
            # small 1-bank (qkv/v/proj/fc2), ctx [65,512]. 2*2+2+2 = 8 banks.
            psum_big = st.enter_context(
                tc.tile_pool(name="psum_big", bufs=2, space="PSUM"))
            psum_small = st.enter_context(
                tc.tile_pool(name="psum_small", bufs=2, space="PSUM"))
            psum_ctx = st.enter_context(
                tc.tile_pool(name="psum_ctx", bufs=2, space="PSUM"))
            h2fm_pool = st.enter_context(tc.tile_pool(name="h2fm", bufs=1))
            g_pool = st.enter_context(tc.tile_pool(name="gfm", bufs=1))
            wfc2_pool = st.enter_context(tc.tile_pool(name="wfc2", bufs=1))
            out_pool = st.enter_context(tc.tile_pool(name="outt", bufs=2))

            x_sb = resid_pool.tile([128, TC, DIM], F32)
            for t in range(TC):
                nc.sync.dma_start(out=x_sb[:, t, :], in_=x_dt[:, t, :])
            ident = const_pool.tile([128, 128], BF16)
            make_identity(nc, ident)
            eps_tile = const_pool.tile([128, 1], F32)
            nc.vector.memset(eps_tile, EPS)
            qkb_pm = const_pool.tile([128, 2 * FC], F32)
            nc.sync.dma_start(out=qkb_pm, in_=qkb_pm_d.ap())
            fc1b_pm = const_pool.tile([128, MC_H], F32)
            nc.sync.dma_start(out=fc1b_pm, in_=fc1b_pm_d.ap())
            vb_bc = const_pool.tile([128, DIM], F32)
            nc.sync.dma_start(out=vb_bc, in_=bcast128(vb_d.ap(), DIM))
            projb_bc = const_pool.tile([128, DIM], F32)
            nc.sync.dma_start(out=projb_bc, in_=bcast128(projb_d.ap(), DIM))
            fc2b_bc = const_pool.tile([128, DIM], F32)
            nc.sync.dma_start(out=fc2b_bc, in_=bcast128(fc2b64_d.ap(), DIM))
            ones_bf = const_pool.tile([128, 128], BF16)
            nc.vector.memset(ones_bf, 1.0)
            expk_tile = const_pool.tile([128, 1], F32)
            nc.vector.memset(expk_tile, float(EXP_K * np.log(2.0)))
            projb_row = const_pool.tile([1, DIM], BF16)
            nc.vector.tensor_copy(out=projb_row, in_=projb_bc[0:1, :])
            fc2b_row = const_pool.tile([1, DIM], BF16)
            nc.vector.tensor_copy(out=fc2b_row, in_=fc2b_bc[0:1, :])

            h2_fm = h2fm_pool.tile([128, FC, N_TOK], BF16, tag="hfm2")
            g_fm = g_pool.tile([128, MC_H, N_TOK], F8)
            wf2 = wfc2_pool.tile([128, MC_H, DIM], F8)

            def ln_transpose(t, dst_fm, evac_act=False):
                """LN of token chunk t + PE-transpose into dst_fm[:, :, t*128:]."""
                h_tile = stat_pool.tile([128, DIM], BF16, tag="h_tile")
                _ln_chunk(nc, stat_pool, eps_tile, x_sb[:, t, :], h_tile)
                tr = psum_big.tile([128, FC, 128], BF16, tag="big")
                for f in range(FC):
                    nc.tensor.transpose(
                        tr[:, f, :], h_tile[:, f * 128:(f + 1) * 128], ident)
                if evac_act:
                    nc.scalar.copy(
                        out=dst_fm[:, :, t * 128:(t + 1) * 128], in_=tr)
                else:
                    nc.vector.tensor_copy(
                        out=dst_fm[:, :, t * 128:(t + 1) * 128], in_=tr)

            wfc1_pool = st.enter_context(tc.tile_pool(name="wfc1", bufs=4))

            def fc1_half(half):
                """fc1 (bf16) for one q-half of the tokens + gelu -> g_fm."""
                for m in range(MC_H):
                    w1t = wfc1_pool.tile([128, FC, 128], BF16, tag="w1t")
                    nc.sync.dma_start(
                        out=w1t, in_=fc1_w3[:, :, m * 128:(m + 1) * 128])
                    ps = psum_small.tile([128, 512], F32, tag="sm")
                    for k in range(FC):
                        nc.tensor.matmul(
                            ps, w1t[:, k, :],
                            h2_fm[:, k, half * 512:(half + 1) * 512],
                            start=(k == 0), stop=(k == FC - 1))
                    nc.scalar.activation(
                        out=g_fm[:, m, half * 512:(half + 1) * 512], in_=ps,
                        func=mybir.ActivationFunctionType.Gelu,
                        bias=fc1b_pm[:, m:m + 1], scale=1.0)

            # ============ attention region (qkv + attention + proj) =========
            with AttnPools(tc) as (qk_pool, v_pool, ctx_pool, wproj_pool):
                qk_fm = qk_pool.tile([128, 2 * FC, N_TOK], BF16)
                # v: [128 key, head, key-chunk, 64+1(den)] padded to VPAD for
                # 16B-aligned DoubleRow pair strides
                v_aug = v_pool.tile([128, HEADS, TC, VPAD], F8)
                ctx_fm = ctx_pool.tile([128, FC, N_TOK], F8)
                wp = wproj_pool.tile([128, FC, DIM], F8)

                with QkvPools(tc) as (hfm_pool, wqkv_pool, wv_pool, exps_pool):
                    h_fm = hfm_pool.tile([128, FC, N_TOK], F8, tag="hfm")

                    # v weights resident: [128, 6, 384] x2
                    wv = [wv_pool.tile([128, FC, 384], F8, tag="wv",
                                       name=f"wv{i}") for i in range(2)]
                    for nv in range(2):
                        nc.sync.dma_start(
                            out=wv[nv],
                            in_=qkv_w3[:, :, 2 * DIM + nv * 384:
                                       2 * DIM + (nv + 1) * 384])

                    # LN1 + v per token chunk (v starts the PE early)
                    for t in range(TC):
                        ln_transpose(t, h_fm)
                        for nv in range(2):
                            ps = psum_small.tile([128, 384], F32, tag="sm")
                            for c in range(FC // 2):
                                nc.tensor.matmul(
                                    ps,
                                    h_fm[:, 2 * c:2 * c + 2,
                                         t * 128:(t + 1) * 128],
                                    wv[nv][:, 2 * c:2 * c + 2, :],
                                    start=(c == 0), stop=(c == FC // 2 - 1),
                                    perf_mode=DR)
                            nc.vector.tensor_tensor(
                                out=v_aug[:, nv * 6:(nv + 1) * 6, t, 0:HD],
                                in0=ps.rearrange("p (h d) -> p h d", d=HD),
                                in1=vb_bc[:, nv * 384:(nv + 1) * 384].rearrange(
                                    "p (h d) -> p h d", d=HD),
                                op=mybir.AluOpType.add)
                    nc.vector.memset(v_aug[:, :, :, HD], 1.0)

                    def emit_qk(m, fast):
                        wt = wqkv_pool.tile([128, FC, 128], F8, tag="wqkv")
                        nc.sync.dma_start(
                            out=wt, in_=qkv_w3[:, :, m * 128:(m + 1) * 128])
                        for q in range(2):
                            ps = psum_small.tile([128, 512], F32, tag="sm")
                            if fast:
                                for c in range(FC // 2):
                                    nc.tensor.matmul(
                                        ps, wt[:, 2 * c:2 * c + 2, :],
                                        h_fm[:, 2 * c:2 * c + 2,
                                             q * 512:(q + 1) * 512],
                                        start=(c == 0),
                                        stop=(c == FC // 2 - 1),
                                        perf_mode=DR)
                            else:
                                # normal-rate fp8: identical math, 2x PE time
                                # deliberately spent inside the ACT-bound exp
                                # phase to keep the HAM clock-gate open
                                for k in range(FC):
                                    nc.tensor.matmul(
                                        ps, wt[:, k, :],
                                        h_fm[:, k, q * 512:(q + 1) * 512],
                                        start=(k == 0), stop=(k == FC - 1))
                            nc.vector.tensor_scalar_add(
                                out=qk_fm[:, m, q * 512:(q + 1) * 512], in0=ps,
                                scalar1=qkb_pm[:, m:m + 1])

                    def emit_s_exp(j):
                        """S + exp for head pair (2j, 2j+1); returns exps tiles
                        exps[ab][kcp] = [128, 2, N_TOK] e4m3 (kc pairs)."""
                        exps = [[None] * (TC // 2) for _ in range(2)]
                        for kcp in range(TC // 2):
                            for ab in range(2):
                                e_t = exps_pool.tile([128, 2, N_TOK], F8,
                                                     tag="exps")
                                exps[ab][kcp] = e_t
                                po = 64 * ab
                                for sub in range(2):
                                    kc = 2 * kcp + sub
                                    sp = psum_big.tile([128, N_TOK], F32,
                                                       tag="big")
                                    for q in range(2):
                                        nc.tensor.matmul(
                                            sp[:, q * 512:(q + 1) * 512],
                                            qk_fm[po:po + 64, 6 + j,
                                                  kc * 128:(kc + 1) * 128],
                                            qk_fm[po:po + 64, j,
                                                  q * 512:(q + 1) * 512],
                                            start=True, stop=True)
                                    nc.scalar.activation(
                                        out=e_t[:, sub, :], in_=sp,
                                        func=mybir.ActivationFunctionType.Exp,
                                        scale=SCALE, bias=expk_tile)
                        return exps

                    def emit_ctx(j, exps):
                        for ab in range(2):
                            h = 2 * j + ab
                            po = 64 * ab
                            for q in range(2):
                                cp = psum_ctx.tile([65, 512], F32, tag="ctx")
                                for kc in range(TC):
                                    nc.tensor.matmul(
                                        cp,
                                        v_aug[:, h, kc, 0:65],
                                        exps[ab][kc // 2][:, kc % 2,
                                                          q * 512:(q + 1) * 512],
                                        start=(kc == 0), stop=(kc == TC - 1))
                                den = dsm_pool.tile([1, 512], F32, tag="den")
                                nc.vector.tensor_copy(out=den,
                                                      in_=cp[64:65, :])
                                scr = dsm_pool.tile([1, 512], F32, tag="scr")
                                rec = dsm_pool.tile([1, 512], F32, tag="rec")
                                nc.vector.reciprocal_approx_accurate(
                                    out=rec, in_=den, scratch=scr)
                                bcd = dsm_pool.tile([128, 512], F32, tag="bcd")
                                nc.gpsimd.partition_broadcast(bcd, rec)
                                if ab == 0:
                                    nc.vector.tensor_tensor(
                                        out=ctx_fm[0:64, j,
                                                   q * 512:(q + 1) * 512],
                                        in0=cp[0:64, :], in1=bcd[0:64, :],
                                        op=mybir.AluOpType.mult)
                                else:
                                    cu = dsm_pool.tile([128, 512], F32,
                                                       tag="cu")
                                    nc.vector.tensor_copy(out=cu[po:po + 64, :],
                                                          in_=cp[0:64, :])
                                    nc.vector.tensor_tensor(
                                        out=ctx_fm[po:po + 64, j,
                                                   q * 512:(q + 1) * 512],
                                        in0=cu[po:po + 64, :],
                                        in1=bcd[po:po + 64, :],
                                        op=mybir.AluOpType.mult)

                    pend = None
                    for j in range(6):
                        emit_qk(j, fast=(j == 0))
                        emit_qk(6 + j, fast=(j == 0))
                        if j == 3:
                            nc.sync.dma_start(out=wp, in_=proj_w3)
                        if j == 4:
                            nc.sync.dma_start(out=wf2, in_=fc2_w3)
                        exps = emit_s_exp(j)
                        if pend is not None:
                            emit_ctx(*pend)
                        pend = (j, exps)
                    emit_ctx(*pend)

                # ---------------- proj + residual + LN2 ----------------
                for t in range(TC):
                    for nv in range(2):
                        ps = psum_small.tile([128, 384], F32, tag="sm")
                        for c in range(FC // 2):
                            nc.tensor.matmul(
                                ps,
                                ctx_fm[:, 2 * c:2 * c + 2,
                                       t * 128:(t + 1) * 128],
                                wp[:, 2 * c:2 * c + 2,
                                   nv * 384:(nv + 1) * 384],
                                start=(c == 0), stop=False, perf_mode=DR)
                        sl = slice(nv * 384, (nv + 1) * 384)
                        nc.tensor.matmul(
                            ps, ones_bf[0:1, :], projb_row[0:1, sl],
                            start=False, stop=True)
                        nc.vector.tensor_add(
                            out=x_sb[:, t, sl], in0=ps, in1=x_sb[:, t, sl])
                    ln_transpose(t, h2_fm, evac_act=True)
                    if t == 3:
                        fc1_half(0)
            fc1_half(1)

            # ---------------- MLP: fc2 (fp8 DR) ----------------
            if True:
                for t in range(TC):
                    o_t = out_pool.tile([128, DIM], F32, tag="outt")
                    for nv in range(2):
                        ps = psum_small.tile([128, 384], F32, tag="sm")
                        for c in range(MC_H // 2):
                            nc.tensor.matmul(
                                ps,
                                g_fm[:, 2 * c:2 * c + 2,
                                     t * 128:(t + 1) * 128],
                                wf2[:, 2 * c:2 * c + 2,
                                    nv * 384:(nv + 1) * 384],
                                start=(c == 0), stop=False, perf_mode=DR)
                        sl = slice(nv * 384, (nv + 1) * 384)
                        nc.tensor.matmul(
                            ps, ones_bf[0:1, :], fc2b_row[0:1, sl],
                            start=False, stop=True)
                        nc.vector.tensor_scalar_mul(
                            out=o_t[:, sl], in0=ps, scalar1=1.0 / FC2_WS)
                    nc.vector.tensor_add(out=o_t, in0=o_t, in1=x_sb[:, t, :])
                    nc.sync.dma_start(out=out_dt[:, t, :], in_=o_t)

    nc.compile()
    return nc


def host_prep(x, ln1_g, ln1_b, qkv_w, proj_w, proj_b, ln2_g, ln2_b,
              fc1_w, fc1_b, fc2_w, fc2_b):
    """Fold LN affine params into weights, pre-transpose, cast to fp8/bf16."""
    import ml_dtypes
    f32 = np.float32
    bf16 = ml_dtypes.bfloat16
    f8 = ml_dtypes.float8_e4m3
    qkv_w = np.asarray(qkv_w, f32)
    qkv_wt = np.ascontiguousarray(
        (qkv_w * np.asarray(ln1_g, f32)[None, :]).T).astype(f8)
    qkv_bias = qkv_w @ np.asarray(ln1_b, f32)
    qkb_pm = np.ascontiguousarray(qkv_bias[:2 * DIM].reshape(2 * FC, 128).T)
    vb = np.ascontiguousarray(qkv_bias[2 * DIM:])
    proj_wt = np.ascontiguousarray(np.asarray(proj_w, f32).T).astype(f8)
    fc1_w = np.asarray(fc1_w, f32)
    fc1_wt = np.ascontiguousarray(
        (fc1_w * np.asarray(ln2_g, f32)[None, :]).T).astype(bf16)
    fc1_bias = fc1_w @ np.asarray(ln2_b, f32) + np.asarray(fc1_b, f32)
    fc1b_pm = np.ascontiguousarray(fc1_bias.reshape(MC_H, 128).T)
    fc2_wt = np.ascontiguousarray(
        np.asarray(fc2_w, f32).T * FC2_WS).astype(f8)
    fc2b64 = np.ascontiguousarray(np.asarray(fc2_b, f32) * FC2_WS)
    return {
        "qkv_wt": qkv_wt, "qkb_pm": qkb_pm, "vb": vb,
        "proj_wt": proj_wt, "projb": np.ascontiguousarray(np.asarray(proj_b, f32)),
        "fc1_wt": fc1_wt, "fc1b_pm": fc1b_pm,
        "fc2_wt": fc2_wt, "fc2b64": fc2b64,
    }


_CACHE = {}


def kernel(x, ln1_g, ln1_b, qkv_w, proj_w, proj_b, ln2_g, ln2_b,
           fc1_w, fc1_b, fc2_w, fc2_b, _want_results=False, **_ignored):
    from concourse.bass_utils import run_bass_kernel_spmd

    x = np.asarray(x, np.float32)
    B = x.shape[0]
    assert B == 8 and x.shape[1] == N_TOK and x.shape[2] == DIM

    w = host_prep(x, ln1_g, ln1_b, qkv_w, proj_w, proj_b, ln2_g, ln2_b,
                  fc1_w, fc1_b, fc2_w, fc2_b)

    if "nc" not in _CACHE:
        _CACHE["nc"] = build_bass()
    nc = _CACHE["nc"]

    in_maps = [dict(w, x=np.ascontiguousarray(x[i])) for i in range(B)]
    res = run_bass_kernel_spmd(nc, in_maps, core_ids=list(range(B)))
    out = np.stack([res.results[i]["out"] for i in range(B)], axis=0)
    if _want_results:
        return out, res
    return out
